# revision 17
# baseline (speedup 1.0000x reference)
"""Trainium2 Bass kernel for the 2-layer hyperbolic (Poincare ball) GCN encoder.

Strategy (8 NeuronCores, SPMD):
  - Nodes sharded across cores (2500 rows/core, padded to 2560 = 20 tiles of 128).
  - Weights replicated (bf16); dense mobius_matvec/mobius_add/logmap0 computed on
    the owned shard with all per-row reductions fused into per-partition scalar
    "grid" tensors of shape [128, T].
  - Per-layer exchange: tangent features (pre-scaled by deg^-0.5 on the source
    side) are AllGathered in bf16 across the 8 cores.
  - Edges partitioned by destination, sorted, grouped into 128-destination
    blocks x 128-edge chunks. Messages fetched with dma_gather (1024 rows per
    instruction); segment-sum on TensorE via 0/1 selection matrices (broadcast
    is_equal) accumulated in PSUM.
  - Destination-side deg^-0.5 and expmap0 fold into one scalar multiply/tile.
"""
import os
import numpy as np
import ml_dtypes

import concourse.bass as bass
import concourse.bacc as bacc
import concourse.tile as tile
import concourse.mybir as mybir
from concourse.bass_utils import run_bass_kernel_spmd
from concourse.masks import make_identity

NCORES = 8
P = 128
GN = 1024            # indices per dma_gather
MN = 1.0 - 4e-3
EPS = 1e-15
ATEPS = 1e-7

f32 = mybir.dt.float32
bf16 = mybir.dt.bfloat16
i16 = mybir.dt.int16
AF = mybir.ActivationFunctionType
OP = mybir.AluOpType

_prog_cache = {}


# ----------------------------------------------------------------- host side

def _np_expmap0(u):
    u = np.asarray(u, np.float32)
    n = max(float(np.linalg.norm(u)), EPS)
    v = (np.tanh(n) * u / n).astype(np.float32)
    nn = max(float(np.linalg.norm(v)), EPS)
    if nn > MN:
        v = (v / nn * MN).astype(np.float32)
    return v


def _host_prep(x, edge_index):
    x = np.asarray(x, np.float32)
    ei = np.asarray(edge_index)
    N, D = x.shape
    assert N % NCORES == 0
    n_loc = N // NCORES
    T = (n_loc + P - 1) // P
    n_pad = T * P
    assert NCORES * n_pad <= 32767, "indices must fit int16"

    loops = np.arange(N, dtype=ei.dtype)
    ei = np.concatenate([ei, np.stack([loops, loops])], axis=1)
    row, col = ei[0], ei[1]
    deg = np.bincount(col, minlength=N).astype(np.float32)
    dis = (deg ** -0.5).astype(np.float32)

    order = np.argsort(col, kind="stable")
    row_s = row[order].astype(np.int64)
    col_s = col[order].astype(np.int64)

    blk = (col_s % n_loc) // P + (col_s // n_loc) * T
    blk_counts = np.bincount(blk, minlength=NCORES * T)
    C = int(np.ceil(blk_counts.max() / P))
    NG = (T * C * P + GN - 1) // GN

    gidx = np.zeros((NCORES, P, T * C), np.int64)
    edst = np.full((NCORES, P, T * C), -1.0, np.float32)
    src_pad = ((row_s // n_loc) * n_pad + row_s % n_loc).astype(np.int64)
    dst_rel = ((col_s % n_loc) % P).astype(np.float32)

    bounds = np.concatenate([[0], np.cumsum(blk_counts)])
    for r in range(NCORES):
        for b in range(T):
            lo, hi = bounds[r * T + b], bounds[r * T + b + 1]
            L = hi - lo
            if L == 0:
                continue
            nchunks = (L + P - 1) // P
            padded = np.zeros(nchunks * P, np.int64)
            padded[:L] = src_pad[lo:hi]
            dpad = np.full(nchunks * P, -1.0, np.float32)
            dpad[:L] = dst_rel[lo:hi]
            cols = b * C + np.arange(nchunks)
            gidx[r][:, cols] = padded.reshape(nchunks, P).T
            edst[r][:, cols] = dpad.reshape(nchunks, P).T

    # linear edge-slot order (slot j*128+p), padded to NG*GN, int16-wrapped
    idx_w = np.zeros((NCORES, 128, NG * (GN // 16)), np.int16)
    for r in range(NCORES):
        lin = np.zeros(NG * GN, np.int64)
        lin[:T * C * P] = gidx[r].T.ravel()
        w = lin.reshape(NG, GN // 16, 16).transpose(2, 0, 1).reshape(16, -1)
        idx_w[r] = np.tile(w.astype(np.int16), (8, 1))

    dis_loc = np.zeros((NCORES, P, T), np.float32)
    for r in range(NCORES):
        d = np.zeros(n_pad, np.float32)
        d[:n_loc] = dis[r * n_loc:(r + 1) * n_loc]
        dis_loc[r] = d.reshape(T, P).T

    x_loc = np.zeros((NCORES, n_pad, D), np.float32)
    for r in range(NCORES):
        x_loc[r, :n_loc] = x[r * n_loc:(r + 1) * n_loc]

    iota = np.tile(np.arange(P, dtype=np.float32)[None, :], (P, 1))
    meta = dict(N=N, D=D, n_loc=n_loc, T=T, C=C, NG=NG, n_pad=n_pad)
    per_core = [dict(x=x_loc[r], dis=dis_loc[r], gidx=idx_w[r],
                     edst=edst[r].astype(ml_dtypes.bfloat16),
                     iota=iota.astype(ml_dtypes.bfloat16))
                for r in range(NCORES)]
    return meta, per_core


# --------------------------------------------------------------- device side

def _build_program(T, C, NG, DC):
    D = DC * P
    NPAD = T * P
    EX = bf16

    nc = bacc.Bacc("TRN2", target_bir_lowering=False, debug=False,
                   num_devices=NCORES, num_swdge_queues=4,
                   dynamic_dma_scratch_size=int(os.environ.get("KSCRATCH", "16384")))

    x_in = nc.dram_tensor("x", [NPAD, D], f32, kind="ExternalInput")
    wt_in = nc.dram_tensor("wt", [2, D, D], bf16, kind="ExternalInput")
    y_in = nc.dram_tensor("y", [2, P, D], f32, kind="ExternalInput")
    iota_in = nc.dram_tensor("iota", [P, P], bf16, kind="ExternalInput")
    dis_in = nc.dram_tensor("dis", [P, T], f32, kind="ExternalInput")
    gidx_in = nc.dram_tensor("gidx", [P, NG * (GN // 16)], i16,
                             kind="ExternalInput")
    edst_in = nc.dram_tensor("edst", [P, T * C], bf16, kind="ExternalInput")
    out_ext = nc.dram_tensor("out", [NPAD, D], f32, kind="ExternalOutput")

    with tile.TileContext(nc) as tc:
        with (
            tc.tile_pool(name="const", bufs=1) as constp,
            tc.tile_pool(name="grid", bufs=1) as gridp,
            tc.tile_pool(name="big", bufs=1) as bigp,
            tc.tile_pool(name="work", bufs=3) as workp,
            tc.tile_pool(name="junk", bufs=3) as junkp,
            tc.tile_pool(name="msgs", bufs=4) as msgp,
            tc.tile_pool(name="sblk", bufs=2) as sblkp,
            tc.tile_pool(name="psum", bufs=2, space="PSUM") as psump,
            tc.tile_pool(name="psag", bufs=3, space="PSUM") as psagp,
            tc.tile_pool(name="dram", bufs=1, space="DRAM") as dramp,
        ):
            # ---- constants ----
            wt_sb = constp.tile([P, 2 * DC * D], bf16, name="wt", tag="wt")
            for l in range(2):
                for k in range(DC):
                    nc.sync.dma_start(
                        out=wt_sb[:, (l * DC + k) * D:(l * DC + k + 1) * D],
                        in_=wt_in[l, k * P:(k + 1) * P, :])
            y_sb = constp.tile([P, 2 * D], f32, name="y", tag="y")
            nc.sync.dma_start(out=y_sb[:, 0:D], in_=y_in[0])
            nc.sync.dma_start(out=y_sb[:, D:2 * D], in_=y_in[1])
            iota_sb = constp.tile([P, P], bf16, name="iota", tag="iota")
            nc.sync.dma_start(out=iota_sb[:], in_=iota_in[:, :])
            ident = constp.tile([P, P], f32, name="ident", tag="ident")
            make_identity(nc, ident[:])
            disg = constp.tile([P, T], f32, name="dis", tag="dis")
            nc.sync.dma_start(out=disg[:], in_=dis_in[:, :])
            gidx_sb = constp.tile([P, NG * (GN // 16)], i16, name="gidx",
                                  tag="gidx")
            nc.sync.dma_start(out=gidx_sb[:], in_=gidx_in[:, :])
            edst_sb = constp.tile([P, T * C], bf16, name="edst", tag="edst")
            nc.sync.dma_start(out=edst_sb[:], in_=edst_in[:, :])

            # ---- persistent big tensors ----
            h_grid = bigp.tile([P, T * D], f32, name="h", tag="h")  # h then u
            agg_grid = bigp.tile([P, T * D], bf16, name="agg", tag="agg")
            hn2 = gridp.tile([P, T], f32, name="hn2", tag="hn2")

            def G(tag):
                return gridp.tile([P, T], f32, name=tag, tag=tag)

            def tsl(t):
                return slice(t * D, (t + 1) * D)

            def artanh2(nm, xx):
                """grid of 2*artanh(clip(xx)), xx >= 0"""
                xcl = G(nm + "_xcl")
                nc.vector.tensor_scalar_min(xcl[:], xx[:], 1.0 - ATEPS)
                a1 = G(nm + "_a1")
                nc.scalar.activation(a1[:], xcl[:], AF.Ln, bias=1.0, scale=1.0)
                omx = G(nm + "_omx")
                nc.vector.tensor_scalar(out=omx[:], in0=xcl[:], scalar1=-1.0,
                                        scalar2=1.0, op0=OP.mult, op1=OP.add)
                a2 = G(nm + "_a2")
                nc.scalar.activation(a2[:], omx[:], AF.Ln)
                at2 = G(nm + "_at2")
                nc.vector.tensor_tensor(out=at2[:], in0=a1[:], in1=a2[:],
                                        op=OP.subtract)
                return at2

            def expmap_scalars(nm, n2, dis_ap):
                n = G(nm + "_n")
                nc.scalar.activation(n[:], n2[:], AF.Sqrt)
                if dis_ap is not None:
                    npr = G(nm + "_npr")
                    nc.vector.tensor_tensor(out=npr[:], in0=n[:], in1=dis_ap,
                                            op=OP.mult)
                else:
                    npr = n
                ng = G(nm + "_ng")
                nc.vector.tensor_scalar_max(ng[:], npr[:], EPS)
                tn = G(nm + "_tn")
                nc.scalar.activation(tn[:], npr[:], AF.Tanh)
                rec = G(nm + "_rec")
                nc.vector.reciprocal(rec[:], ng[:])
                sc0 = G(nm + "_sc0")
                nc.vector.tensor_tensor(out=sc0[:], in0=tn[:], in1=rec[:],
                                        op=OP.mult)
                tng = G(nm + "_tng")
                nc.vector.tensor_scalar_max(tng[:], tn[:], EPS)
                trec = G(nm + "_trec")
                nc.vector.reciprocal(trec[:], tng[:])
                ps = G(nm + "_ps")
                nc.vector.tensor_scalar(out=ps[:], in0=trec[:], scalar1=MN,
                                        scalar2=1.0, op0=OP.mult, op1=OP.min)
                sig = G(nm + "_sig")
                nc.vector.tensor_tensor(out=sig[:], in0=sc0[:], in1=ps[:],
                                        op=OP.mult)
                if dis_ap is not None:
                    sig2 = G(nm + "_sig2")
                    nc.vector.tensor_tensor(out=sig2[:], in0=sig[:],
                                            in1=dis_ap, op=OP.mult)
                    sig = sig2
                tnm = G(nm + "_tnm")
                nc.vector.tensor_scalar_min(tnm[:], tn[:], MN)
                nc.vector.tensor_tensor(out=hn2[:], in0=tnm[:], in1=tnm[:],
                                        op=OP.mult)
                return sig

            # ================= init: h = expmap0(x) =================
            n2i = G("n2i")
            for t in range(T):
                nc.sync.dma_start(out=h_grid[:, tsl(t)],
                                  in_=x_in[t * P:(t + 1) * P, :])
                jj = junkp.tile([P, D], f32, name="junk", tag="junk")
                nc.scalar.activation(jj[:], h_grid[:, tsl(t)], AF.Square,
                                     accum_out=n2i[:, t:t + 1])
            sig0 = expmap_scalars("em0", n2i, None)
            for t in range(T):
                eng = nc.gpsimd if t % 2 == 0 else nc.vector
                eng.tensor_scalar_mul(h_grid[:, tsl(t)],
                                      h_grid[:, tsl(t)],
                                      sig0[:, t:t + 1])

            mxn2_g = [G("mxn2_0"), G("mxn2_1")]

            def emit_pass1(l, t):
                pt = psump.tile([P, D], f32, name="pt", tag="pt")
                for k in range(DC):
                    nc.tensor.transpose(
                        out=pt[:, k * P:(k + 1) * P],
                        in_=h_grid[:, t * D + k * P: t * D + (k + 1) * P],
                        identity=ident[:])
                hT = workp.tile([P, D], bf16, name="hT", tag="hT")
                nc.vector.tensor_copy(hT[:], pt[:])
                pm = psump.tile([P, D], f32, name="pm", tag="pm")
                for k in range(DC):
                    nc.tensor.matmul(
                        pm[:],
                        lhsT=hT[:, k * P:(k + 1) * P],
                        rhs=wt_sb[:, (l * DC + k) * D:(l * DC + k + 1) * D],
                        start=(k == 0), stop=(k == DC - 1))
                nc.scalar.copy(agg_grid[:, tsl(t)], pm[:])
                jj = junkp.tile([P, D], f32, name="junk", tag="junk")
                nc.scalar.activation(jj[:], pm[:], AF.Square,
                                     accum_out=mxn2_g[l][:, t:t + 1])

            # ================= layers =================
            for l in range(2):
                y_ap = y_sb[:, l * D:(l + 1) * D]
                ts_loc = dramp.tile([NPAD, D], EX, name="ts_loc%d" % l,
                                    tag="ts_loc%d" % l)
                ts_full = dramp.tile([NCORES * NPAD, D], EX,
                                     addr_space="Shared",
                                     name="ts_full%d" % l, tag="ts_full%d" % l)
                jy = junkp.tile([P, D], f32, name="junk", tag="junk")
                y2col = gridp.tile([P, 1], f32, name="y2col", tag="y2col")
                nc.scalar.activation(jy[:], y_ap, AF.Square, accum_out=y2col[:])

                mxn2 = mxn2_g[l]
                # ---- phase A pass 1 (layer 1's tiles are emitted inside
                # layer 0's phase-B block loop for cross-layer overlap) ----
                if l == 0:
                    for t in range(T):
                        emit_pass1(0, t)

                # ---- stage-1 scalars ----
                xn = G("xn")
                nc.scalar.activation(xn[:], hn2[:], AF.Sqrt)
                xng = G("xng")
                nc.vector.tensor_scalar_max(xng[:], xn[:], EPS)
                xrec = G("xrec")
                nc.vector.reciprocal(xrec[:], xng[:])
                at2 = artanh2("s1", xn)
                rr2 = G("rr2")
                nc.vector.tensor_tensor(out=rr2[:], in0=at2[:], in1=xrec[:],
                                        op=OP.mult)
                mxn = G("mxn")
                nc.scalar.activation(mxn[:], mxn2[:], AF.Sqrt)
                mxng = G("mxng")
                nc.vector.tensor_scalar_max(mxng[:], mxn[:], EPS)
                mrec = G("mrec")
                nc.vector.reciprocal(mrec[:], mxng[:])
                cc = G("cc")
                nc.vector.scalar_tensor_tensor(out=cc[:], in0=mxn[:],
                                               scalar=0.5, in1=rr2[:],
                                               op0=OP.mult, op1=OP.mult)
                tch = G("tch")
                nc.scalar.activation(tch[:], cc[:], AF.Tanh)
                tcg = G("tcg")
                nc.vector.tensor_scalar_max(tcg[:], tch[:], EPS)
                tcrec = G("tcrec")
                nc.vector.reciprocal(tcrec[:], tcg[:])
                psA = G("psA")
                nc.vector.tensor_scalar(out=psA[:], in0=tcrec[:], scalar1=MN,
                                        scalar2=1.0, op0=OP.mult, op1=OP.min)
                sp0 = G("sp0")
                nc.vector.tensor_tensor(out=sp0[:], in0=tch[:], in1=mrec[:],
                                        op=OP.mult)
                spg = G("spg")
                nc.vector.tensor_tensor(out=spg[:], in0=sp0[:], in1=psA[:],
                                        op=OP.mult)
                tcm = G("tcm")
                nc.vector.tensor_scalar_min(tcm[:], tch[:], MN)
                x2 = G("x2")
                nc.vector.tensor_tensor(out=x2[:], in0=tcm[:], in1=tcm[:],
                                        op=OP.mult)

                # ---- pass 2: v = sp * mx (in place), xy ----
                xy = G("xy")
                for t in range(T):
                    mxs = agg_grid[:, tsl(t)]
                    nc.vector.tensor_scalar_mul(mxs, mxs, spg[:, t:t + 1])
                    jx = junkp.tile([P, D], f32, name="junk", tag="junk")
                    nc.vector.tensor_tensor(out=jx[:], in0=mxs, in1=y_ap,
                                            op=OP.mult)
                    nc.vector.tensor_reduce(out=xy[:, t:t + 1], in_=jx[:],
                                            axis=mybir.AxisListType.X,
                                            op=OP.add)

                # ---- stage-2 scalars ----
                t0 = G("t0")
                nc.vector.tensor_scalar(out=t0[:], in0=xy[:], scalar1=2.0,
                                        scalar2=1.0, op0=OP.mult, op1=OP.add)
                ag = G("ag")
                nc.vector.tensor_scalar_add(ag[:], t0[:], y2col[:, 0:1])
                d0 = G("d0")
                nc.vector.tensor_scalar_mul(d0[:], x2[:], y2col[:, 0:1])
                d1 = G("d1")
                nc.vector.tensor_tensor(out=d1[:], in0=d0[:], in1=t0[:],
                                        op=OP.add)
                dg = G("dg")
                nc.vector.tensor_scalar_max(dg[:], d1[:], EPS)
                dinv = G("dinv")
                nc.vector.reciprocal(dinv[:], dg[:])
                alpha = G("alpha")
                nc.vector.tensor_tensor(out=alpha[:], in0=ag[:], in1=dinv[:],
                                        op=OP.mult)
                bsc = G("bsc")
                nc.vector.tensor_scalar(out=bsc[:], in0=x2[:], scalar1=-1.0,
                                        scalar2=1.0, op0=OP.mult, op1=OP.add)
                beta = G("beta")
                nc.vector.tensor_tensor(out=beta[:], in0=bsc[:], in1=dinv[:],
                                        op=OP.mult)

                # ---- pass 3: u = alpha*v + beta*y (into h_grid), un2 ----
                un2 = G("un2")
                for t in range(T):
                    t1 = workp.tile([P, D], f32, name="t1", tag="t1")
                    nc.gpsimd.tensor_scalar_mul(t1[:], y_ap, beta[:, t:t + 1])
                    us = h_grid[:, tsl(t)]
                    nc.vector.scalar_tensor_tensor(
                        out=us, in0=agg_grid[:, tsl(t)],
                        scalar=alpha[:, t:t + 1], in1=t1[:],
                        op0=OP.mult, op1=OP.add)
                    ju = junkp.tile([P, D], f32, name="junk", tag="junk")
                    nc.scalar.activation(ju[:], us, AF.Square,
                                         accum_out=un2[:, t:t + 1])

                # ---- stage-3 scalars: gamma ----
                un = G("un")
                nc.scalar.activation(un[:], un2[:], AF.Sqrt)
                ung = G("ung")
                nc.vector.tensor_scalar_max(ung[:], un[:], EPS)
                urec = G("urec")
                nc.vector.reciprocal(urec[:], ung[:])
                h2n = G("h2n")
                nc.vector.tensor_scalar_min(h2n[:], un[:], MN)
                at2u = artanh2("s3", h2n)
                h2ng = G("h2ng")
                nc.vector.tensor_scalar_max(h2ng[:], h2n[:], EPS)
                hrec = G("hrec")
                nc.vector.reciprocal(hrec[:], h2ng[:])
                lam2 = G("lam2")
                nc.vector.tensor_tensor(out=lam2[:], in0=at2u[:], in1=hrec[:],
                                        op=OP.mult)
                pst = G("pst")
                nc.vector.tensor_scalar(out=pst[:], in0=urec[:], scalar1=MN,
                                        scalar2=1.0, op0=OP.mult, op1=OP.min)
                gm0 = G("gm0")
                nc.vector.scalar_tensor_tensor(out=gm0[:], in0=lam2[:],
                                               scalar=0.5, in1=pst[:],
                                               op0=OP.mult, op1=OP.mult)
                gam = G("gam")
                nc.vector.tensor_tensor(out=gam[:], in0=gm0[:], in1=disg[:],
                                        op=OP.mult)

                # ---- ts tiles out + allgather ----
                for t in range(T):
                    tst = workp.tile([P, D], EX, name="tst", tag="tst")
                    eng = nc.gpsimd if t % 2 == 0 else nc.vector
                    eng.tensor_scalar_mul(tst[:], h_grid[:, tsl(t)],
                                          gam[:, t:t + 1])
                    nc.sync.dma_start(out=ts_loc[t * P:(t + 1) * P, :],
                                      in_=tst[:])
                nc.gpsimd.collective_compute(
                    "AllGather", OP.bypass,
                    replica_groups=[list(range(NCORES))],
                    ins=[ts_loc.opt()], outs=[ts_full.opt()])

                # ---- phase B: dma_gather stream + segment-sum ----
                an2 = G("an2")
                CPG = GN // P  # chunks per gather
                mtiles = {}

                def Gc(tag):
                    return gridp.tile([P, 1], f32, name=tag, tag=tag)

                def expmap_col(nm, n2_ap, dis_ap, hn2_ap):
                    n = Gc(nm + "_n")
                    nc.scalar.activation(n[:], n2_ap, AF.Sqrt)
                    npr = Gc(nm + "_npr")
                    nc.vector.tensor_tensor(out=npr[:], in0=n[:], in1=dis_ap,
                                            op=OP.mult)
                    ng = Gc(nm + "_ng")
                    nc.vector.tensor_scalar_max(ng[:], npr[:], EPS)
                    tn = Gc(nm + "_tn")
                    nc.scalar.activation(tn[:], npr[:], AF.Tanh)
                    rec = Gc(nm + "_rec")
                    nc.vector.reciprocal(rec[:], ng[:])
                    sc0 = Gc(nm + "_sc0")
                    nc.vector.tensor_tensor(out=sc0[:], in0=tn[:], in1=rec[:],
                                            op=OP.mult)
                    tng = Gc(nm + "_tng")
                    nc.vector.tensor_scalar_max(tng[:], tn[:], EPS)
                    trec = Gc(nm + "_trec")
                    nc.vector.reciprocal(trec[:], tng[:])
                    ps = Gc(nm + "_ps")
                    nc.vector.tensor_scalar(out=ps[:], in0=trec[:], scalar1=MN,
                                            scalar2=1.0, op0=OP.mult,
                                            op1=OP.min)
                    sig = Gc(nm + "_sig")
                    nc.vector.tensor_tensor(out=sig[:], in0=sc0[:], in1=ps[:],
                                            op=OP.mult)
                    sig2 = Gc(nm + "_sig2")
                    nc.vector.tensor_tensor(out=sig2[:], in0=sig[:],
                                            in1=dis_ap, op=OP.mult)
                    tnm = Gc(nm + "_tnm")
                    nc.vector.tensor_scalar_min(tnm[:], tn[:], MN)
                    nc.vector.tensor_tensor(out=hn2_ap, in0=tnm[:], in1=tnm[:],
                                            op=OP.mult)
                    return sig2

                for b in range(T):
                    S = sblkp.tile([P, C * P], EX, name="S", tag="S")
                    nc.vector.tensor_tensor(
                        out=S[:].rearrange("p (c j) -> p c j", c=C),
                        in0=edst_sb[:, b * C:(b + 1) * C].to_broadcast(
                            [P, C, P]),
                        in1=iota_sb[:].rearrange("p (o j) -> p o j", o=1)
                            .to_broadcast([P, C, P]),
                        op=OP.is_equal)
                    pa = psagp.tile([P, D], f32, name="pa", tag="pa")
                    for c in range(C):
                        j = b * C + c
                        g, s = divmod(j, CPG)
                        if g not in mtiles:
                            m = msgp.tile([P, CPG * D], EX, name="m", tag="m")
                            nc.gpsimd.dma_gather(
                                m[:].rearrange("p (c e) -> p c e", c=CPG),
                                ts_full,
                                gidx_sb[:, g * (GN // 16):(g + 1) * (GN // 16)],
                                GN, GN, D, queue_num=g % 4)
                            mtiles = {g: m}
                        m = mtiles[g]
                        nc.tensor.matmul(pa[:],
                                         lhsT=S[:, c * P:(c + 1) * P],
                                         rhs=m[:, s * D:(s + 1) * D],
                                         start=(c == 0), stop=(c == C - 1))
                    jj = junkp.tile([P, D], f32, name="junk", tag="junk")
                    nc.scalar.activation(jj[:], pa[:], AF.Square,
                                         accum_out=an2[:, b:b + 1])
                    sigb = expmap_col("emC", an2[:, b:b + 1],
                                      disg[:, b:b + 1], hn2[:, b:b + 1])
                    nc.vector.tensor_scalar_mul(h_grid[:, tsl(b)], pa[:],
                                                sigb[:, 0:1])
                    if l == 0:
                        emit_pass1(1, b)
                    else:
                        nc.sync.dma_start(out=out_ext[b * P:(b + 1) * P, :],
                                          in_=h_grid[:, tsl(b)])

    nc.compile()
    return nc


def _get_program(T, C, NG, DC):
    key = (T, C, NG, DC)
    if key not in _prog_cache:
        _prog_cache[key] = _build_program(T, C, NG, DC)
    return _prog_cache[key]


# ----------------------------------------------------------------- entry

def run(inputs, trace=False, trace_kwargs=None):
    x = np.asarray(inputs["x"], np.float32)
    ei = np.asarray(inputs["edge_index"])
    W1 = np.asarray(inputs["W1"], np.float32)
    b1 = np.asarray(inputs["b1"], np.float32)
    W2 = np.asarray(inputs["W2"], np.float32)
    b2 = np.asarray(inputs["b2"], np.float32)
    N, D = x.shape
    assert D % P == 0
    meta, per_core = _host_prep(x, ei)
    T, C, NG, DC = meta["T"], meta["C"], meta["NG"], D // P
    n_loc = meta["n_loc"]

    wt = np.stack([np.ascontiguousarray(W1.T), np.ascontiguousarray(W2.T)])
    wt = wt.astype(ml_dtypes.bfloat16)
    y = np.stack([np.tile(_np_expmap0(b1)[None, :], (P, 1)),
                  np.tile(_np_expmap0(b2)[None, :], (P, 1))])

    nc = _get_program(T, C, NG, DC)
    in_maps = []
    for r in range(NCORES):
        m = dict(per_core[r])
        m["wt"] = wt
        m["y"] = y
        in_maps.append(m)

    kwargs = {}
    if trace:
        kwargs = dict(trace=True, trace_kwargs=trace_kwargs or {})
    res = run_bass_kernel_spmd(nc, in_maps, list(range(NCORES)), **kwargs)
    out = np.concatenate(
        [np.asarray(res.results[r]["out"])[:n_loc] for r in range(NCORES)],
        axis=0)
    return out, res


def kernel(**inputs):
    out, _ = run(inputs)
    return out


# revision 18
# speedup vs baseline: 1.0056x; 1.0056x over previous
"""Trainium2 Bass kernel for the 2-layer hyperbolic (Poincare ball) GCN encoder.

Strategy (8 NeuronCores, SPMD):
  - Nodes sharded across cores (2500 rows/core, padded to 2560 = 20 tiles of 128).
  - Weights replicated (bf16); dense mobius_matvec/mobius_add/logmap0 computed on
    the owned shard with all per-row reductions fused into per-partition scalar
    "grid" tensors of shape [128, T].
  - Per-layer exchange: tangent features (pre-scaled by deg^-0.5 on the source
    side) are AllGathered in bf16 across the 8 cores.
  - Edges partitioned by destination, sorted, grouped into 128-destination
    blocks x 128-edge chunks. Messages fetched with dma_gather (1024 rows per
    instruction); segment-sum on TensorE via 0/1 selection matrices (broadcast
    is_equal) accumulated in PSUM.
  - Destination-side deg^-0.5 and expmap0 fold into one scalar multiply/tile.
"""
import os
import numpy as np
import ml_dtypes

import concourse.bass as bass
import concourse.bacc as bacc
import concourse.tile as tile
import concourse.mybir as mybir
from concourse.bass_utils import run_bass_kernel_spmd
from concourse.masks import make_identity

NCORES = 8
P = 128
GN = 1024            # indices per dma_gather
MN = 1.0 - 4e-3
EPS = 1e-15
ATEPS = 1e-7

f32 = mybir.dt.float32
bf16 = mybir.dt.bfloat16
i16 = mybir.dt.int16
AF = mybir.ActivationFunctionType
OP = mybir.AluOpType

_prog_cache = {}


# ----------------------------------------------------------------- host side

def _np_expmap0(u):
    u = np.asarray(u, np.float32)
    n = max(float(np.linalg.norm(u)), EPS)
    v = (np.tanh(n) * u / n).astype(np.float32)
    nn = max(float(np.linalg.norm(v)), EPS)
    if nn > MN:
        v = (v / nn * MN).astype(np.float32)
    return v


def _host_prep(x, edge_index):
    x = np.asarray(x, np.float32)
    ei = np.asarray(edge_index)
    N, D = x.shape
    assert N % NCORES == 0
    n_loc = N // NCORES
    T = (n_loc + P - 1) // P
    n_pad = T * P
    assert NCORES * n_pad <= 32767, "indices must fit int16"

    loops = np.arange(N, dtype=ei.dtype)
    ei = np.concatenate([ei, np.stack([loops, loops])], axis=1)
    row, col = ei[0], ei[1]
    deg = np.bincount(col, minlength=N).astype(np.float32)
    dis = (deg ** -0.5).astype(np.float32)

    order = np.argsort(col, kind="stable")
    row_s = row[order].astype(np.int64)
    col_s = col[order].astype(np.int64)

    blk = (col_s % n_loc) // P + (col_s // n_loc) * T
    blk_counts = np.bincount(blk, minlength=NCORES * T)
    C = int(np.ceil(blk_counts.max() / P))
    NG = (T * C * P + GN - 1) // GN

    gidx = np.zeros((NCORES, P, T * C), np.int64)
    edst = np.full((NCORES, P, T * C), -1.0, np.float32)
    src_pad = ((row_s // n_loc) * n_pad + row_s % n_loc).astype(np.int64)
    dst_rel = ((col_s % n_loc) % P).astype(np.float32)

    bounds = np.concatenate([[0], np.cumsum(blk_counts)])
    for r in range(NCORES):
        for b in range(T):
            lo, hi = bounds[r * T + b], bounds[r * T + b + 1]
            L = hi - lo
            if L == 0:
                continue
            nchunks = (L + P - 1) // P
            padded = np.zeros(nchunks * P, np.int64)
            padded[:L] = src_pad[lo:hi]
            dpad = np.full(nchunks * P, -1.0, np.float32)
            dpad[:L] = dst_rel[lo:hi]
            cols = b * C + np.arange(nchunks)
            gidx[r][:, cols] = padded.reshape(nchunks, P).T
            edst[r][:, cols] = dpad.reshape(nchunks, P).T

    # linear edge-slot order (slot j*128+p), padded to NG*GN, int16-wrapped
    idx_w = np.zeros((NCORES, 128, NG * (GN // 16)), np.int16)
    for r in range(NCORES):
        lin = np.zeros(NG * GN, np.int64)
        lin[:T * C * P] = gidx[r].T.ravel()
        w = lin.reshape(NG, GN // 16, 16).transpose(2, 0, 1).reshape(16, -1)
        idx_w[r] = np.tile(w.astype(np.int16), (8, 1))

    dis_loc = np.zeros((NCORES, P, T), np.float32)
    for r in range(NCORES):
        d = np.zeros(n_pad, np.float32)
        d[:n_loc] = dis[r * n_loc:(r + 1) * n_loc]
        dis_loc[r] = d.reshape(T, P).T

    x_loc = np.zeros((NCORES, n_pad, D), np.float32)
    for r in range(NCORES):
        x_loc[r, :n_loc] = x[r * n_loc:(r + 1) * n_loc]

    iota = np.tile(np.arange(P, dtype=np.float32)[None, :], (P, 1))
    meta = dict(N=N, D=D, n_loc=n_loc, T=T, C=C, NG=NG, n_pad=n_pad)
    per_core = [dict(x=x_loc[r], dis=dis_loc[r], gidx=idx_w[r],
                     edst=edst[r].astype(ml_dtypes.bfloat16),
                     iota=iota.astype(ml_dtypes.bfloat16))
                for r in range(NCORES)]
    return meta, per_core


# --------------------------------------------------------------- device side

def _build_program(T, C, NG, DC):
    D = DC * P
    NPAD = T * P
    EX = bf16

    nc = bacc.Bacc("TRN2", target_bir_lowering=False, debug=False,
                   num_devices=NCORES, num_swdge_queues=4,
                   dynamic_dma_scratch_size=int(os.environ.get("KSCRATCH", "16384")))

    x_in = nc.dram_tensor("x", [NPAD, D], f32, kind="ExternalInput")
    wt_in = nc.dram_tensor("wt", [2, D, D], bf16, kind="ExternalInput")
    y_in = nc.dram_tensor("y", [2, P, D], f32, kind="ExternalInput")
    iota_in = nc.dram_tensor("iota", [P, P], bf16, kind="ExternalInput")
    dis_in = nc.dram_tensor("dis", [P, T], f32, kind="ExternalInput")
    gidx_in = nc.dram_tensor("gidx", [P, NG * (GN // 16)], i16,
                             kind="ExternalInput")
    edst_in = nc.dram_tensor("edst", [P, T * C], bf16, kind="ExternalInput")
    out_ext = nc.dram_tensor("out", [NPAD, D], f32, kind="ExternalOutput")

    with tile.TileContext(nc) as tc:
        with (
            tc.tile_pool(name="const", bufs=1) as constp,
            tc.tile_pool(name="grid", bufs=1) as gridp,
            tc.tile_pool(name="big", bufs=1) as bigp,
            tc.tile_pool(name="work", bufs=3) as workp,
            tc.tile_pool(name="junk", bufs=3) as junkp,
            tc.tile_pool(name="msgs", bufs=4) as msgp,
            tc.tile_pool(name="sblk", bufs=2) as sblkp,
            tc.tile_pool(name="psum", bufs=2, space="PSUM") as psump,
            tc.tile_pool(name="psag", bufs=3, space="PSUM") as psagp,
            tc.tile_pool(name="dram", bufs=1, space="DRAM") as dramp,
        ):
            # ---- constants ----
            wt_sb = constp.tile([P, 2 * DC * D], bf16, name="wt", tag="wt")
            for l in range(2):
                for k in range(DC):
                    nc.sync.dma_start(
                        out=wt_sb[:, (l * DC + k) * D:(l * DC + k + 1) * D],
                        in_=wt_in[l, k * P:(k + 1) * P, :])
            y_sb = constp.tile([P, 2 * D], f32, name="y", tag="y")
            nc.sync.dma_start(out=y_sb[:, 0:D], in_=y_in[0])
            nc.sync.dma_start(out=y_sb[:, D:2 * D], in_=y_in[1])
            iota_sb = constp.tile([P, P], bf16, name="iota", tag="iota")
            nc.sync.dma_start(out=iota_sb[:], in_=iota_in[:, :])
            ident = constp.tile([P, P], f32, name="ident", tag="ident")
            make_identity(nc, ident[:])
            disg = constp.tile([P, T], f32, name="dis", tag="dis")
            nc.sync.dma_start(out=disg[:], in_=dis_in[:, :])
            gidx_sb = constp.tile([P, NG * (GN // 16)], i16, name="gidx",
                                  tag="gidx")
            nc.sync.dma_start(out=gidx_sb[:], in_=gidx_in[:, :])
            edst_sb = constp.tile([P, T * C], bf16, name="edst", tag="edst")
            nc.sync.dma_start(out=edst_sb[:], in_=edst_in[:, :])

            # ---- persistent big tensors ----
            h_grid = bigp.tile([P, T * D], f32, name="h", tag="h")  # h then u
            agg_grid = bigp.tile([P, T * D], bf16, name="agg", tag="agg")
            hn2 = gridp.tile([P, T], f32, name="hn2", tag="hn2")

            def G(tag):
                return gridp.tile([P, T], f32, name=tag, tag=tag)

            def tsl(t):
                return slice(t * D, (t + 1) * D)

            def artanh2(nm, xx):
                """grid of 2*artanh(clip(xx)), xx >= 0"""
                xcl = G(nm + "_xcl")
                nc.vector.tensor_scalar_min(xcl[:], xx[:], 1.0 - ATEPS)
                a1 = G(nm + "_a1")
                nc.scalar.activation(a1[:], xcl[:], AF.Ln, bias=1.0, scale=1.0)
                omx = G(nm + "_omx")
                nc.vector.tensor_scalar(out=omx[:], in0=xcl[:], scalar1=-1.0,
                                        scalar2=1.0, op0=OP.mult, op1=OP.add)
                a2 = G(nm + "_a2")
                nc.scalar.activation(a2[:], omx[:], AF.Ln)
                at2 = G(nm + "_at2")
                nc.vector.tensor_tensor(out=at2[:], in0=a1[:], in1=a2[:],
                                        op=OP.subtract)
                return at2

            def expmap_scalars(nm, n2, dis_ap):
                n = G(nm + "_n")
                nc.scalar.activation(n[:], n2[:], AF.Sqrt)
                if dis_ap is not None:
                    npr = G(nm + "_npr")
                    nc.vector.tensor_tensor(out=npr[:], in0=n[:], in1=dis_ap,
                                            op=OP.mult)
                else:
                    npr = n
                ng = G(nm + "_ng")
                nc.vector.tensor_scalar_max(ng[:], npr[:], EPS)
                tn = G(nm + "_tn")
                nc.scalar.activation(tn[:], npr[:], AF.Tanh)
                rec = G(nm + "_rec")
                nc.vector.reciprocal(rec[:], ng[:])
                sc0 = G(nm + "_sc0")
                nc.vector.tensor_tensor(out=sc0[:], in0=tn[:], in1=rec[:],
                                        op=OP.mult)
                tng = G(nm + "_tng")
                nc.vector.tensor_scalar_max(tng[:], tn[:], EPS)
                trec = G(nm + "_trec")
                nc.vector.reciprocal(trec[:], tng[:])
                ps = G(nm + "_ps")
                nc.vector.tensor_scalar(out=ps[:], in0=trec[:], scalar1=MN,
                                        scalar2=1.0, op0=OP.mult, op1=OP.min)
                sig = G(nm + "_sig")
                nc.vector.tensor_tensor(out=sig[:], in0=sc0[:], in1=ps[:],
                                        op=OP.mult)
                if dis_ap is not None:
                    sig2 = G(nm + "_sig2")
                    nc.vector.tensor_tensor(out=sig2[:], in0=sig[:],
                                            in1=dis_ap, op=OP.mult)
                    sig = sig2
                tnm = G(nm + "_tnm")
                nc.vector.tensor_scalar_min(tnm[:], tn[:], MN)
                nc.vector.tensor_tensor(out=hn2[:], in0=tnm[:], in1=tnm[:],
                                        op=OP.mult)
                return sig

            # ================= init: h = expmap0(x) =================
            n2i = G("n2i")
            for t in range(T):
                nc.sync.dma_start(out=h_grid[:, tsl(t)],
                                  in_=x_in[t * P:(t + 1) * P, :])
                jj = junkp.tile([P, D], f32, name="junk", tag="junk")
                nc.scalar.activation(jj[:], h_grid[:, tsl(t)], AF.Square,
                                     accum_out=n2i[:, t:t + 1])
            sig0 = expmap_scalars("em0", n2i, None)
            for t in range(T):
                eng = nc.gpsimd if t % 2 == 0 else nc.vector
                eng.tensor_scalar_mul(h_grid[:, tsl(t)],
                                      h_grid[:, tsl(t)],
                                      sig0[:, t:t + 1])

            mxn2_g = [G("mxn2_0"), G("mxn2_1")]

            def emit_pass1(l, t):
                pt = psump.tile([P, D], f32, name="pt", tag="pt")
                for k in range(DC):
                    nc.tensor.transpose(
                        out=pt[:, k * P:(k + 1) * P],
                        in_=h_grid[:, t * D + k * P: t * D + (k + 1) * P],
                        identity=ident[:])
                hT = workp.tile([P, D], bf16, name="hT", tag="hT")
                nc.vector.tensor_copy(hT[:], pt[:])
                pm = psump.tile([P, D], f32, name="pm", tag="pm")
                for k in range(DC):
                    nc.tensor.matmul(
                        pm[:],
                        lhsT=hT[:, k * P:(k + 1) * P],
                        rhs=wt_sb[:, (l * DC + k) * D:(l * DC + k + 1) * D],
                        start=(k == 0), stop=(k == DC - 1))
                nc.scalar.copy(agg_grid[:, tsl(t)], pm[:])
                jj = junkp.tile([P, D], f32, name="junk", tag="junk")
                nc.scalar.activation(jj[:], pm[:], AF.Square,
                                     accum_out=mxn2_g[l][:, t:t + 1])

            # ================= layers =================
            for l in range(2):
                y_ap = y_sb[:, l * D:(l + 1) * D]
                ts_loc = dramp.tile([NPAD, D], EX, name="ts_loc%d" % l,
                                    tag="ts_loc%d" % l)
                ts_full = dramp.tile([NCORES * NPAD, D], EX,
                                     addr_space="Shared",
                                     name="ts_full%d" % l, tag="ts_full%d" % l)
                jy = junkp.tile([P, D], f32, name="junk", tag="junk")
                y2col = gridp.tile([P, 1], f32, name="y2col", tag="y2col")
                nc.scalar.activation(jy[:], y_ap, AF.Square, accum_out=y2col[:])

                mxn2 = mxn2_g[l]
                # ---- phase A pass 1 (layer 1's tiles are emitted inside
                # layer 0's phase-B block loop for cross-layer overlap) ----
                if l == 0:
                    for t in range(T):
                        emit_pass1(0, t)

                # ---- stage-1 scalars ----
                xn = G("xn")
                nc.scalar.activation(xn[:], hn2[:], AF.Sqrt)
                xng = G("xng")
                nc.vector.tensor_scalar_max(xng[:], xn[:], EPS)
                xrec = G("xrec")
                nc.vector.reciprocal(xrec[:], xng[:])
                at2 = artanh2("s1", xn)
                rr2 = G("rr2")
                nc.vector.tensor_tensor(out=rr2[:], in0=at2[:], in1=xrec[:],
                                        op=OP.mult)
                mxn = G("mxn")
                nc.scalar.activation(mxn[:], mxn2[:], AF.Sqrt)
                mxng = G("mxng")
                nc.vector.tensor_scalar_max(mxng[:], mxn[:], EPS)
                mrec = G("mrec")
                nc.vector.reciprocal(mrec[:], mxng[:])
                cc = G("cc")
                nc.vector.scalar_tensor_tensor(out=cc[:], in0=mxn[:],
                                               scalar=0.5, in1=rr2[:],
                                               op0=OP.mult, op1=OP.mult)
                tch = G("tch")
                nc.scalar.activation(tch[:], cc[:], AF.Tanh)
                tcg = G("tcg")
                nc.vector.tensor_scalar_max(tcg[:], tch[:], EPS)
                tcrec = G("tcrec")
                nc.vector.reciprocal(tcrec[:], tcg[:])
                psA = G("psA")
                nc.vector.tensor_scalar(out=psA[:], in0=tcrec[:], scalar1=MN,
                                        scalar2=1.0, op0=OP.mult, op1=OP.min)
                sp0 = G("sp0")
                nc.vector.tensor_tensor(out=sp0[:], in0=tch[:], in1=mrec[:],
                                        op=OP.mult)
                spg = G("spg")
                nc.vector.tensor_tensor(out=spg[:], in0=sp0[:], in1=psA[:],
                                        op=OP.mult)
                tcm = G("tcm")
                nc.vector.tensor_scalar_min(tcm[:], tch[:], MN)
                x2 = G("x2")
                nc.vector.tensor_tensor(out=x2[:], in0=tcm[:], in1=tcm[:],
                                        op=OP.mult)

                # ---- pass 2: v = sp * mx (in place), xy ----
                xy = G("xy")
                for t in range(T):
                    mxs = agg_grid[:, tsl(t)]
                    nc.vector.tensor_scalar_mul(mxs, mxs, spg[:, t:t + 1])
                    jx = junkp.tile([P, D], f32, name="junk", tag="junk")
                    nc.vector.tensor_tensor(out=jx[:], in0=mxs, in1=y_ap,
                                            op=OP.mult)
                    nc.vector.tensor_reduce(out=xy[:, t:t + 1], in_=jx[:],
                                            axis=mybir.AxisListType.X,
                                            op=OP.add)

                # ---- stage-2 scalars ----
                t0 = G("t0")
                nc.vector.tensor_scalar(out=t0[:], in0=xy[:], scalar1=2.0,
                                        scalar2=1.0, op0=OP.mult, op1=OP.add)
                ag = G("ag")
                nc.vector.tensor_scalar_add(ag[:], t0[:], y2col[:, 0:1])
                d0 = G("d0")
                nc.vector.tensor_scalar_mul(d0[:], x2[:], y2col[:, 0:1])
                d1 = G("d1")
                nc.vector.tensor_tensor(out=d1[:], in0=d0[:], in1=t0[:],
                                        op=OP.add)
                dg = G("dg")
                nc.vector.tensor_scalar_max(dg[:], d1[:], EPS)
                dinv = G("dinv")
                nc.vector.reciprocal(dinv[:], dg[:])
                alpha = G("alpha")
                nc.vector.tensor_tensor(out=alpha[:], in0=ag[:], in1=dinv[:],
                                        op=OP.mult)
                bsc = G("bsc")
                nc.vector.tensor_scalar(out=bsc[:], in0=x2[:], scalar1=-1.0,
                                        scalar2=1.0, op0=OP.mult, op1=OP.add)
                beta = G("beta")
                nc.vector.tensor_tensor(out=beta[:], in0=bsc[:], in1=dinv[:],
                                        op=OP.mult)

                # ---- pass 3: u = alpha*v + beta*y (into h_grid), un2 ----
                un2 = G("un2")
                for t in range(T):
                    t1 = workp.tile([P, D], f32, name="t1", tag="t1")
                    nc.gpsimd.tensor_scalar_mul(t1[:], y_ap, beta[:, t:t + 1])
                    us = h_grid[:, tsl(t)]
                    nc.vector.scalar_tensor_tensor(
                        out=us, in0=agg_grid[:, tsl(t)],
                        scalar=alpha[:, t:t + 1], in1=t1[:],
                        op0=OP.mult, op1=OP.add)
                    ju = junkp.tile([P, D], f32, name="junk", tag="junk")
                    nc.scalar.activation(ju[:], us, AF.Square,
                                         accum_out=un2[:, t:t + 1])

                # ---- stage-3 scalars: gamma ----
                un = G("un")
                nc.scalar.activation(un[:], un2[:], AF.Sqrt)
                ung = G("ung")
                nc.vector.tensor_scalar_max(ung[:], un[:], EPS)
                urec = G("urec")
                nc.vector.reciprocal(urec[:], ung[:])
                h2n = G("h2n")
                nc.vector.tensor_scalar_min(h2n[:], un[:], MN)
                at2u = artanh2("s3", h2n)
                h2ng = G("h2ng")
                nc.vector.tensor_scalar_max(h2ng[:], h2n[:], EPS)
                hrec = G("hrec")
                nc.vector.reciprocal(hrec[:], h2ng[:])
                lam2 = G("lam2")
                nc.vector.tensor_tensor(out=lam2[:], in0=at2u[:], in1=hrec[:],
                                        op=OP.mult)
                pst = G("pst")
                nc.vector.tensor_scalar(out=pst[:], in0=urec[:], scalar1=MN,
                                        scalar2=1.0, op0=OP.mult, op1=OP.min)
                gm0 = G("gm0")
                nc.vector.scalar_tensor_tensor(out=gm0[:], in0=lam2[:],
                                               scalar=0.5, in1=pst[:],
                                               op0=OP.mult, op1=OP.mult)
                gam = G("gam")
                nc.vector.tensor_tensor(out=gam[:], in0=gm0[:], in1=disg[:],
                                        op=OP.mult)

                # ---- ts tiles out + allgather ----
                for t in range(T):
                    tst = workp.tile([P, D], EX, name="tst", tag="tst")
                    eng = nc.gpsimd if t % 2 == 0 else nc.vector
                    eng.tensor_scalar_mul(tst[:], h_grid[:, tsl(t)],
                                          gam[:, t:t + 1])
                    nc.sync.dma_start(out=ts_loc[t * P:(t + 1) * P, :],
                                      in_=tst[:])
                nc.gpsimd.collective_compute(
                    "AllGather", OP.bypass,
                    replica_groups=[list(range(NCORES))],
                    ins=[ts_loc.opt()], outs=[ts_full.opt()])

                # ---- phase B: dma_gather stream + segment-sum ----
                an2 = G("an2")
                CPG = GN // P  # chunks per gather
                mtiles = {}

                def Gc(tag):
                    return gridp.tile([P, 1], f32, name=tag, tag=tag)

                def expmap_col(nm, n2_ap, dis_ap, hn2_ap):
                    n = Gc(nm + "_n")
                    nc.scalar.activation(n[:], n2_ap, AF.Sqrt)
                    npr = Gc(nm + "_npr")
                    nc.vector.tensor_tensor(out=npr[:], in0=n[:], in1=dis_ap,
                                            op=OP.mult)
                    ng = Gc(nm + "_ng")
                    nc.vector.tensor_scalar_max(ng[:], npr[:], EPS)
                    tn = Gc(nm + "_tn")
                    nc.scalar.activation(tn[:], npr[:], AF.Tanh)
                    rec = Gc(nm + "_rec")
                    nc.vector.reciprocal(rec[:], ng[:])
                    sc0 = Gc(nm + "_sc0")
                    nc.vector.tensor_tensor(out=sc0[:], in0=tn[:], in1=rec[:],
                                            op=OP.mult)
                    tng = Gc(nm + "_tng")
                    nc.vector.tensor_scalar_max(tng[:], tn[:], EPS)
                    trec = Gc(nm + "_trec")
                    nc.vector.reciprocal(trec[:], tng[:])
                    ps = Gc(nm + "_ps")
                    nc.vector.tensor_scalar(out=ps[:], in0=trec[:], scalar1=MN,
                                            scalar2=1.0, op0=OP.mult,
                                            op1=OP.min)
                    sig = Gc(nm + "_sig")
                    nc.vector.tensor_tensor(out=sig[:], in0=sc0[:], in1=ps[:],
                                            op=OP.mult)
                    sig2 = Gc(nm + "_sig2")
                    nc.vector.tensor_tensor(out=sig2[:], in0=sig[:],
                                            in1=dis_ap, op=OP.mult)
                    tnm = Gc(nm + "_tnm")
                    nc.vector.tensor_scalar_min(tnm[:], tn[:], MN)
                    nc.vector.tensor_tensor(out=hn2_ap, in0=tnm[:], in1=tnm[:],
                                            op=OP.mult)
                    return sig2

                for b in range(T):
                    S = sblkp.tile([P, C * P], EX, name="S", tag="S")
                    nc.vector.tensor_tensor(
                        out=S[:].rearrange("p (c j) -> p c j", c=C),
                        in0=edst_sb[:, b * C:(b + 1) * C].to_broadcast(
                            [P, C, P]),
                        in1=iota_sb[:].rearrange("p (o j) -> p o j", o=1)
                            .to_broadcast([P, C, P]),
                        op=OP.is_equal)
                    pa = psagp.tile([P, D], f32, name="pa", tag="pa")
                    for c in range(C):
                        j = b * C + c
                        g, s = divmod(j, CPG)
                        if g not in mtiles:
                            m = msgp.tile([P, CPG * D], EX, name="m", tag="m")
                            nc.gpsimd.dma_gather(
                                m[:].rearrange("p (c e) -> p c e", c=CPG),
                                ts_full,
                                gidx_sb[:, g * (GN // 16):(g + 1) * (GN // 16)],
                                GN, GN, D, queue_num=g % 4)
                            mtiles = {g: m}
                        m = mtiles[g]
                        nc.tensor.matmul(pa[:],
                                         lhsT=S[:, c * P:(c + 1) * P],
                                         rhs=m[:, s * D:(s + 1) * D],
                                         start=(c == 0), stop=(c == C - 1))
                    jj = junkp.tile([P, D], f32, name="junk", tag="junk")
                    nc.scalar.activation(jj[:], pa[:], AF.Square,
                                         accum_out=an2[:, b:b + 1])
                    sigb = expmap_col("emC", an2[:, b:b + 1],
                                      disg[:, b:b + 1], hn2[:, b:b + 1])
                    nc.vector.tensor_scalar_mul(h_grid[:, tsl(b)], pa[:],
                                                sigb[:, 0:1])
                    if l == 0:
                        if b >= 3:
                            emit_pass1(1, b - 3)
                        if b == T - 1:
                            for tt in range(T - 3, T):
                                emit_pass1(1, tt)
                    else:
                        nc.sync.dma_start(out=out_ext[b * P:(b + 1) * P, :],
                                          in_=h_grid[:, tsl(b)])

    nc.compile()
    return nc


def _get_program(T, C, NG, DC):
    key = (T, C, NG, DC)
    if key not in _prog_cache:
        _prog_cache[key] = _build_program(T, C, NG, DC)
    return _prog_cache[key]


# ----------------------------------------------------------------- entry

def run(inputs, trace=False, trace_kwargs=None):
    x = np.asarray(inputs["x"], np.float32)
    ei = np.asarray(inputs["edge_index"])
    W1 = np.asarray(inputs["W1"], np.float32)
    b1 = np.asarray(inputs["b1"], np.float32)
    W2 = np.asarray(inputs["W2"], np.float32)
    b2 = np.asarray(inputs["b2"], np.float32)
    N, D = x.shape
    assert D % P == 0
    meta, per_core = _host_prep(x, ei)
    T, C, NG, DC = meta["T"], meta["C"], meta["NG"], D // P
    n_loc = meta["n_loc"]

    wt = np.stack([np.ascontiguousarray(W1.T), np.ascontiguousarray(W2.T)])
    wt = wt.astype(ml_dtypes.bfloat16)
    y = np.stack([np.tile(_np_expmap0(b1)[None, :], (P, 1)),
                  np.tile(_np_expmap0(b2)[None, :], (P, 1))])

    nc = _get_program(T, C, NG, DC)
    in_maps = []
    for r in range(NCORES):
        m = dict(per_core[r])
        m["wt"] = wt
        m["y"] = y
        in_maps.append(m)

    kwargs = {}
    if trace:
        kwargs = dict(trace=True, trace_kwargs=trace_kwargs or {})
    res = run_bass_kernel_spmd(nc, in_maps, list(range(NCORES)), **kwargs)
    out = np.concatenate(
        [np.asarray(res.results[r]["out"])[:n_loc] for r in range(NCORES)],
        axis=0)
    return out, res


def kernel(**inputs):
    out, _ = run(inputs)
    return out
